# revision 6
# baseline (speedup 1.0000x reference)
"""Based linear-attention (parallel form) on 8 TRN2 NeuronCores.

Sharding: core c handles batch b = c // 4 and head-group g = c % 4
(3 of 12 heads).  Wq/Wk/Wv are column-split by head, Wo row-split; each
core emits a partial [L, D] output and the host sums the 4 partials per
batch (the row-parallel all-reduce is done as part of unsharding).

Device algorithm per core (all matmuls contract on the partition dim):
  hsT [D, L] (host-transposed)  ->  qT/kT [96, L] (heads padded to
  32-row groups so the 3 heads' sT matmuls land in distinct PE row
  groups and run concurrently),  v [L, 384].
  Per l-strip of 512 and head h:  sT = k q^T (psum), attnT =
  ((sT+1)^2 + 1) masked causally, oT_h accumulates v_h^T attnT,
  z row 32h accumulates ones^T attnT.  Then 1/z is broadcast across
  partitions with a K=1 matmul and oT is normalized, and the output
  projection accumulates the 3 heads into psum.
"""

import sys

sys.path.insert(0, "/opt/trn_rl_repo")

from contextlib import ExitStack

import ml_dtypes
import numpy as np

import concourse.bass as bass
import concourse.tile as tile
from concourse import bacc, mybir
from concourse.bass_utils import run_bass_kernel_spmd

B, L, D = 2, 2048, 1536
H, FDIM, HD = 12, 16, 128
NH = 3          # heads per core
GQK = 96        # padded q or k rows (3 heads x 32)
DV = NH * HD    # 384 v/o columns per core
SW = 512        # l-strip width
P = 128
NK = D // P     # 12 contraction tiles
NM = L // P     # 16 m/l tiles
NJ = L // SW    # 4 l strips
NDC = D // SW   # 3 output column strips

DT = mybir.dt.bfloat16
NPDT = ml_dtypes.bfloat16
F32 = mybir.dt.float32
F32R = mybir.dt.float32r

_ADD = mybir.AluOpType.add
_MULT = mybir.AluOpType.mult
_SQUARE = mybir.ActivationFunctionType.Square


def _build():
    nc = bacc.Bacc("TRN2", target_bir_lowering=False, debug=False, num_devices=8)

    hsT = nc.dram_tensor("hsT", [D, L], DT, kind="ExternalInput").ap()
    wq = nc.dram_tensor("wq", [D, GQK], DT, kind="ExternalInput").ap()
    wk = nc.dram_tensor("wk", [D, GQK], DT, kind="ExternalInput").ap()
    wv = nc.dram_tensor("wv", [D, DV], DT, kind="ExternalInput").ap()
    wo = nc.dram_tensor("wo", [DV, D], DT, kind="ExternalInput").ap()
    masks = nc.dram_tensor("masks", [4 * P, SW], DT, kind="ExternalInput").ap()
    onesr = nc.dram_tensor("onesr", [GQK, P], F32R, kind="ExternalInput").ap()
    out = nc.dram_tensor("out", [L, D], F32, kind="ExternalOutput").ap()

    with tile.TileContext(nc, trace_sim=False) as tc, ExitStack() as ctx:
        cpool = ctx.enter_context(tc.tile_pool(name="consts", bufs=1))
        wq_sb = cpool.tile([P, NK * GQK], DT, tag="wq")
        wk_sb = cpool.tile([P, NK * GQK], DT, tag="wk")
        wv_sb = cpool.tile([P, NK * DV], DT, tag="wv")
        wo_sb = cpool.tile([P, NH * D], DT, tag="wo")
        masks_sb = cpool.tile([P, 4 * SW], DT, tag="masks")
        ones_col = cpool.tile([P, 1], DT, tag="ones_col")
        ones_row = cpool.tile([GQK, P], F32R, tag="ones_row")
        for k in range(NK):
            nc.sync.dma_start(wq_sb[:, k * GQK : (k + 1) * GQK], wq[k * P : (k + 1) * P, :])
            nc.sync.dma_start(wk_sb[:, k * GQK : (k + 1) * GQK], wk[k * P : (k + 1) * P, :])
            nc.sync.dma_start(wv_sb[:, k * DV : (k + 1) * DV], wv[k * P : (k + 1) * P, :])
        for dd in range(NH):
            nc.sync.dma_start(wo_sb[:, dd * D : (dd + 1) * D], wo[dd * P : (dd + 1) * P, :])
        for c in range(4):
            nc.sync.dma_start(masks_sb[:, c * SW : (c + 1) * SW], masks[c * P : (c + 1) * P, :])
        nc.vector.memset(ones_col[:], 1.0)
        nc.sync.dma_start(ones_row[:], onesr[:])

        hpool = ctx.enter_context(tc.tile_pool(name="hsT", bufs=NK))
        hs_t = []
        for k in range(NK):
            t = hpool.tile([P, L], DT, tag="hsT")
            nc.sync.dma_start(t[:], hsT[k * P : (k + 1) * P, :])
            hs_t.append(t)

        qkv_pool = ctx.enter_context(tc.tile_pool(name="qkv", bufs=1))
        qT_sb = qkv_pool.tile([GQK, L], DT, tag="qT")
        kT_sb = qkv_pool.tile([GQK, L], DT, tag="kT")
        v_sb = qkv_pool.tile([P, NM * DV], DT, tag="v")

        # ---- projections ----
        with tc.tile_pool(name="ps_proj", bufs=4, space="PSUM") as ps_proj:
            for j in range(NJ):
                qp = ps_proj.tile([GQK, SW], F32, tag="p")
                kp = ps_proj.tile([GQK, SW], F32, tag="p")
                for k in range(NK):
                    nc.tensor.matmul(
                        qp[:], wq_sb[:, k * GQK : (k + 1) * GQK],
                        hs_t[k][:, j * SW : (j + 1) * SW],
                        start=(k == 0), stop=(k == NK - 1))
                for k in range(NK):
                    nc.tensor.matmul(
                        kp[:], wk_sb[:, k * GQK : (k + 1) * GQK],
                        hs_t[k][:, j * SW : (j + 1) * SW],
                        start=(k == 0), stop=(k == NK - 1))
                nc.vector.tensor_copy(qT_sb[:, j * SW : (j + 1) * SW], qp[:])
                nc.vector.tensor_copy(kT_sb[:, j * SW : (j + 1) * SW], kp[:])
            for mt in range(NM):
                vp = ps_proj.tile([P, DV], F32, tag="p")
                for k in range(NK):
                    nc.tensor.matmul(
                        vp[:], hs_t[k][:, mt * P : (mt + 1) * P],
                        wv_sb[:, k * DV : (k + 1) * DV],
                        start=(k == 0), stop=(k == NK - 1))
                nc.vector.tensor_copy(v_sb[:, mt * DV : (mt + 1) * DV], vp[:])

        opool = ctx.enter_context(tc.tile_pool(name="oT", bufs=1))
        oT_sb = [opool.tile([P, L], DT, tag=f"oT{h}", name=f"oT{h}") for h in range(NH)]
        zr_sb = opool.tile([GQK, L], F32R, tag="zr")

        # ---- attention strips ----
        with tc.tile_pool(name="ps_sT", bufs=4, space="PSUM") as ps_sT, \
             tc.tile_pool(name="ps_oT", bufs=NH, space="PSUM") as ps_oT, \
             tc.tile_pool(name="ps_z", bufs=1, space="PSUM") as ps_z, \
             tc.tile_pool(name="attnT", bufs=12) as apool:
            for j in range(NJ):
                nim = 4 * (j + 1)
                otp = [ps_oT.tile([P, SW], F32, tag="o", name=f"otp{j}_{hh}") for hh in range(NH)]
                zp = ps_z.tile([GQK, SW], F32, tag="z")
                for im in range(nim):
                    c = im - 4 * j
                    for h in range(NH):
                        r0 = 32 * h
                        stp = ps_sT.tile([P, SW], F32, tag="s")
                        nc.tensor.matmul(
                            stp[:], kT_sb[r0 : r0 + FDIM, im * P : (im + 1) * P],
                            qT_sb[r0 : r0 + FDIM, j * SW : (j + 1) * SW],
                            start=True, stop=True)
                        att = apool.tile([P, SW], DT, tag="a")
                        nc.scalar.activation(att[:], stp[:], _SQUARE, bias=1.0, scale=1.0)
                        if c >= 0:
                            nc.vector.scalar_tensor_tensor(
                                att[:], att[:], 1.0, masks_sb[:, c * SW : (c + 1) * SW],
                                op0=_ADD, op1=_MULT)
                        else:
                            nc.vector.tensor_scalar_add(att[:], att[:], 1.0)
                        nc.tensor.matmul(
                            otp[h][:], v_sb[:, im * DV + h * HD : im * DV + (h + 1) * HD],
                            att[:], start=(im == 0), stop=(im == nim - 1))
                        nc.tensor.matmul(
                            zp[r0 : r0 + 1, :], ones_col[:], att[:],
                            start=(im == 0), stop=(im == nim - 1))
                for h in range(NH):
                    nc.vector.tensor_copy(oT_sb[h][:, j * SW : (j + 1) * SW], otp[h][:])
                    with nc.allow_low_precision(reason="float32r is f32-width storage"):
                        nc.vector.reciprocal(
                            zr_sb[32 * h : 32 * h + 1, j * SW : (j + 1) * SW],
                            zp[32 * h : 32 * h + 1, :])

        # ---- normalize + output projection ----
        with tc.tile_pool(name="ps_bc", bufs=2, space="PSUM") as ps_bc, \
             tc.tile_pool(name="ps_out", bufs=2, space="PSUM") as ps_out, \
             tc.tile_pool(name="obuf", bufs=4) as obuf:
            for h in range(NH):
                r0 = 32 * h
                for j in range(NJ):
                    bc = ps_bc.tile([P, SW], F32, tag="bc")
                    nc.tensor.matmul(
                        bc[:], ones_row[r0 : r0 + 1, :],
                        zr_sb[r0 : r0 + 1, j * SW : (j + 1) * SW],
                        start=True, stop=True)
                    nc.vector.tensor_mul(
                        oT_sb[h][:, j * SW : (j + 1) * SW],
                        oT_sb[h][:, j * SW : (j + 1) * SW], bc[:])
            for lt in range(NM):
                for dc in range(NDC):
                    op = ps_out.tile([P, SW], F32, tag="op")
                    for h in range(NH):
                        nc.tensor.matmul(
                            op[:], oT_sb[h][:, lt * P : (lt + 1) * P],
                            wo_sb[:, h * D + dc * SW : h * D + (dc + 1) * SW],
                            start=(h == 0), stop=(h == NH - 1))
                    ob = obuf.tile([P, SW], F32, tag="ob")
                    nc.vector.tensor_copy(ob[:], op[:])
                    nc.sync.dma_start(out[lt * P : (lt + 1) * P, dc * SW : (dc + 1) * SW], ob[:])

    nc.compile()
    return nc


def _host_inputs(hidden_states, Wq, Wk, Wv, Wo):
    """Shard + lay out the full inputs into 8 per-core in_maps."""
    scale = FDIM ** -0.5
    mask = np.zeros((4 * P, SW), dtype=np.float32)
    for c in range(4):
        p = np.arange(P)[:, None] + 128 * c
        f = np.arange(SW)[None, :]
        mask[c * P : (c + 1) * P, :] = (p <= f).astype(np.float32)

    in_maps = []
    for core in range(8):
        b, g = divmod(core, 4)
        heads = range(NH * g, NH * (g + 1))
        wq_pad = np.zeros((D, GQK), dtype=np.float32)
        wk_pad = np.zeros((D, GQK), dtype=np.float32)
        for i, h in enumerate(heads):
            wq_pad[:, 32 * i : 32 * i + FDIM] = Wq[:, FDIM * h : FDIM * (h + 1)] * scale
            wk_pad[:, 32 * i : 32 * i + FDIM] = Wk[:, FDIM * h : FDIM * (h + 1)]
        in_maps.append({
            "hsT": np.ascontiguousarray(hidden_states[b].T).astype(NPDT),
            "wq": wq_pad.astype(NPDT),
            "wk": wk_pad.astype(NPDT),
            "wv": np.ascontiguousarray(Wv[:, HD * NH * g : HD * NH * (g + 1)]).astype(NPDT),
            "wo": np.ascontiguousarray(Wo[HD * NH * g : HD * NH * (g + 1), :]).astype(NPDT),
            "masks": mask.astype(NPDT),
            "onesr": np.ones((GQK, P), dtype=np.float32),
        })
    return in_maps


_NC = None


def _get_nc():
    global _NC
    if _NC is None:
        _NC = _build()
    return _NC


def run(hidden_states, Wq, Wk, Wv, Wo, trace=False, **trace_kwargs):
    nc = _get_nc()
    in_maps = _host_inputs(hidden_states, Wq, Wk, Wv, Wo)
    res = run_bass_kernel_spmd(nc, in_maps, core_ids=list(range(8)),
                               trace=trace, **trace_kwargs)
    out = np.zeros((B, L, D), dtype=np.float32)
    for core in range(8):
        out[core // 4] += res.results[core]["out"]
    return out, res


def kernel(hidden_states, Wq, Wk, Wv, Wo):
    out, _ = run(np.asarray(hidden_states, dtype=np.float32),
                 np.asarray(Wq, dtype=np.float32),
                 np.asarray(Wk, dtype=np.float32),
                 np.asarray(Wv, dtype=np.float32),
                 np.asarray(Wo, dtype=np.float32))
    return out
